# revision 5
# baseline (speedup 1.0000x reference)
"""Trainium2 Bass kernel for the ragged-sequence attention problem.

Math (per batch row):
    u      = tanh(h @ W.T + b)          h: [L, H]
    s      = u @ u_s                    masked to length, then softmax
    v      = sum_l alpha_l * h_l

Strategy: pure data parallel over the batch axis, 8 NeuronCores x 32 rows.
Per core, for each batch row:
  - DMA h tiles [128, 240] f32, cast to bf16 (DVE), append a ones column
    (used to accumulate the softmax denominator inside the v matmul).
  - PE-transpose h tiles into ht [240ish, L] bf16 (contraction over H needs
    H on partitions).
  - u-matmul: stationary W.T chunks, moving ht, N=512 groups -> psum [o, l].
  - tanh+bias on ScalarE straight out of PSUM into bf16.
  - scores: stationary u_s column (1-col weight load), moving u tiles;
    col-tiled (tile_position) so the 4 L-groups land on psum partitions
    0/32/64/96 and evacuate in ONE strided DVE copy.
  - reshape scores to [128, 16] via tiny SBUF->SBUF DMAs, exp (ScalarE),
    multiply by a host-precomputed length mask (DVE).
  - v-matmul: stationary weight column w[:, g], moving h bf16 tiles
    (with the ones column -> out[240] holds the denominator).
Host divides v_acc by the denominator and concatenates the 8 shards.
"""

import sys

import numpy as np

sys.path.insert(0, "/opt/trn_rl_repo")

import concourse.bass as bass  # noqa: E402
import concourse.mybir as mybir  # noqa: E402
import concourse.tile as tile  # noqa: E402
from concourse.masks import make_identity  # noqa: E402
from concourse.vector_clock import ScopedClock  # noqa: E402
from concourse.bass_utils import run_bass_kernel_spmd  # noqa: E402
import bass_rust as _br  # noqa: E402

N_CORES = 8
B, L, H = 256, 2048, 240
BPC = B // N_CORES        # batch rows per core
NT = L // 128             # 16 l-tiles of 128
NG = 4                    # l-groups of 512
GSZ = 512
H0, H1 = 128, 112         # H split across partitions
HB = H + 1                # h tile width incl. ones column
F32 = mybir.dt.float32
BF16 = mybir.dt.bfloat16
AF = mybir.ActivationFunctionType


_MAXW = 1  # sync waits kept on an instruction; the rest move to nops


class _TC(tile.TileContext):
    """Walrus in this container caps sync-wait commands per instruction
    ("Too many sync wait commands"), but Tile freely attaches one wait per
    producer semaphore.  After scheduling, hoist excess waits onto dedicated
    single-wait nops inserted just before the instruction on its engine."""

    def schedule_and_allocate(self, validate_deps=False):
        ret = super().schedule_and_allocate(validate_deps)
        self._split_excess_waits()
        return ret

    def _split_excess_waits(self):
        nc = self.nc
        n_split = 0
        for fn in nc.m.functions:
            for bb in fn.blocks:
                insts = bb.instructions
                i = 0
                while i < len(insts):
                    inst = insts[i]
                    si = getattr(inst, "sync_info", None)
                    waits = list(si.on_wait) if si is not None else []
                    if len(waits) > _MAXW:
                        si.on_wait = waits[-_MAXW:]
                        inst.sync_info = si
                        for w in waits[:-_MAXW]:
                            nop = mybir.InstNoOp(
                                name=f"waitsplit-{n_split}", ins=[], outs=[])
                            n_split += 1
                            nop.engine = inst.engine
                            nop.sync_info = _br.SyncInfo(
                                on_wait=[w], on_update=[])
                            nc.register_instruction(nop, overwrite=True)
                            insts.insert(i, nop)
                            i += 1
                    i += 1


def build():
    nc = bass.Bass("TRN2", target_bir_lowering=False, debug=False,
                   num_devices=N_CORES)
    h_d = nc.declare_dram_parameter("h", [BPC, L, H], F32, isOutput=False)
    wt_d = nc.declare_dram_parameter("wt", [H, H], F32, isOutput=False)
    us_d = nc.declare_dram_parameter("usT", [H, BPC], F32, isOutput=False)
    b_d = nc.declare_dram_parameter("bias", [H, 1], F32, isOutput=False)
    m_d = nc.declare_dram_parameter("mask", [BPC, 128, NT], F32, isOutput=False)
    o_d = nc.declare_dram_parameter("out", [BPC, HB], F32, isOutput=True)

    with _TC(nc) as tc:
        with (
            tc.tile_pool(name="consts", bufs=1) as cp,
            tc.tile_pool(name="hf", bufs=2) as hfp,
            tc.tile_pool(name="hbf", bufs=2) as hbfp,
            tc.tile_pool(name="ht", bufs=2) as htp,
            tc.tile_pool(name="ut", bufs=2) as utp,
            tc.tile_pool(name="small", bufs=2) as sp,
            tc.tile_pool(name="pt", bufs=1, space="PSUM") as ptp,
            tc.tile_pool(name="pu", bufs=2, space="PSUM") as pup,
            tc.tile_pool(name="psv", bufs=1, space="PSUM") as psvp,
            tc.tile_pool(name="dscr", bufs=2, space="DRAM") as dp,
        ):
            ident = cp.tile([128, 128], BF16)
            make_identity(nc, ident[:])

            wtf0 = cp.tile([H0, H], F32)
            wtf1 = cp.tile([H1, H], F32)
            nc.sync.dma_start(wtf0[:], wt_d.ap()[0:H0, :])
            nc.sync.dma_start(wtf1[:], wt_d.ap()[H0:H, :])
            wtb0 = cp.tile([H0, H], BF16)
            wtb1 = cp.tile([H1, H], BF16)
            nc.vector.tensor_copy(wtb0[:], wtf0[:])
            nc.vector.tensor_copy(wtb1[:], wtf1[:])

            usf0 = cp.tile([H0, BPC], F32)
            usf1 = cp.tile([H1, BPC], F32)
            nc.sync.dma_start(usf0[:], us_d.ap()[0:H0, :])
            nc.sync.dma_start(usf1[:], us_d.ap()[H0:H, :])
            usb0 = cp.tile([H0, BPC], BF16)
            usb1 = cp.tile([H1, BPC], BF16)
            nc.vector.tensor_copy(usb0[:], usf0[:])
            nc.vector.tensor_copy(usb1[:], usf1[:])

            b0 = cp.tile([H0, 1], F32)
            b1 = cp.tile([H1, 1], F32)
            nc.sync.dma_start(b0[:], b_d.ap()[0:H0, :])
            nc.sync.dma_start(b1[:], b_d.ap()[H0:H, :])

            for b in range(BPC):
                hf = hfp.tile([128, NT * H], F32, tag="hf")
                hview = h_d.ap()[b].rearrange("(t p) c -> p t c", p=128)
                hfv = hf[:].rearrange("p (t c) -> p t c", c=H)
                for q in range(4):
                    nc.sync.dma_start(hfv[:, q * 4:(q + 1) * 4, :],
                                      hview[:, q * 4:(q + 1) * 4, :])

                hbf = hbfp.tile([128, NT * HB], BF16, tag="hbf")
                hbv = hbf[:].rearrange("p (t c) -> p t c", c=HB)
                nc.vector.tensor_copy(hbv[:, :, 0:H], hfv)
                nc.gpsimd.memset(hbv[:, :, H:HB], 1.0)

                msk = sp.tile([128, NT], F32, tag="msk")
                nc.sync.dma_start(msk[:], m_d.ap()[b])

                ht0 = htp.tile([H0, L], BF16, tag="ht0")
                ht1 = htp.tile([H1, L], BF16, tag="ht1")
                ps = psvp.tile([128, GSZ], F32, tag="ps")

                for g in range(NG):
                    pt0 = ptp.tile([H0, GSZ], BF16, tag="pt0")
                    pt1 = ptp.tile([H1, GSZ], BF16, tag="pt1")
                    for t4 in range(4):
                        t = g * 4 + t4
                        base = t * HB
                        nc.tensor.transpose(
                            pt0[:, t4 * 128:(t4 + 1) * 128],
                            hbf[:, base:base + H0], ident[:])
                        nc.tensor.transpose(
                            pt1[:, t4 * 128:(t4 + 1) * 128],
                            hbf[:, base + H0:base + H], ident[:])
                    gs = slice(g * GSZ, (g + 1) * GSZ)
                    nc.vector.tensor_copy(ht0[:, gs], pt0[:])
                    nc.vector.tensor_copy(ht1[:, gs], pt1[:])

                    pu0 = pup.tile([H0, GSZ], F32, tag="pu0")
                    pu1 = pup.tile([H1, GSZ], F32, tag="pu1")
                    nc.tensor.matmul(pu0[:], wtb0[:, 0:H0], ht0[:, gs],
                                     start=True, stop=False)
                    nc.tensor.matmul(pu0[:], wtb1[:, 0:H0], ht1[:, gs],
                                     start=False, stop=True)
                    nc.tensor.matmul(pu1[:], wtb0[:, H0:H], ht0[:, gs],
                                     start=True, stop=False)
                    nc.tensor.matmul(pu1[:], wtb1[:, H0:H], ht1[:, gs],
                                     start=False, stop=True)

                    ut0 = utp.tile([H0, GSZ], BF16, tag="ut0")
                    ut1 = utp.tile([H1, GSZ], BF16, tag="ut1")
                    nc.scalar.activation(ut0[:], pu0[:], AF.Tanh, bias=b0[:])
                    nc.scalar.activation(ut1[:], pu1[:], AF.Tanh, bias=b1[:])

                    nc.tensor.matmul(ps[32 * g:32 * g + 1, :],
                                     usb0[:, b:b + 1], ut0[:],
                                     start=True, stop=False,
                                     tile_position=(0, 32 * g))
                    nc.tensor.matmul(ps[32 * g:32 * g + 1, :],
                                     usb1[:, b:b + 1], ut1[:],
                                     start=False, stop=True,
                                     tile_position=(0, 32 * g))

                s4 = sp.tile([128, GSZ], F32, tag="s4")
                nc.vector.tensor_copy(s4[0:97, :], ps[0:97, :])

                dscr = dp.tile([NG, GSZ], F32, tag="dscr")
                for g in range(NG):
                    nc.sync.dma_start(dscr[g:g + 1, :],
                                      s4[32 * g:32 * g + 1, :])
                scr = sp.tile([128, NT], F32, tag="scr")
                nc.sync.dma_start(
                    scr[:].rearrange("p (g c) -> p g c", g=NG),
                    dscr[:].rearrange("g (c p) -> p g c", p=128))

                e = sp.tile([128, NT], F32, tag="e")
                nc.scalar.activation(e[:], scr[:], AF.Exp)
                w = sp.tile([128, NT], BF16, tag="w")
                nc.vector.tensor_mul(w[:], e[:], msk[:])

                pv = psvp.tile([1, HB], F32, tag="pv")
                for t in range(NT):
                    nc.tensor.matmul(pv[:], w[:, t:t + 1],
                                     hbf[:, t * HB:(t + 1) * HB],
                                     start=(t == 0), stop=(t == NT - 1))

                orow = sp.tile([1, HB], F32, tag="orow")
                nc.vector.tensor_copy(orow[:], pv[:])
                nc.sync.dma_start(o_d.ap()[b:b + 1, :], orow[:])

    return nc


_NC_CACHE = None


def _get_nc():
    global _NC_CACHE
    if _NC_CACHE is None:
        _NC_CACHE = build()
    return _NC_CACHE


def _prep_in_maps(short_perference, current_perference, W, bvec, length_input):
    h = np.asarray(short_perference, dtype=np.float32)[0]      # [B, L, H]
    us = np.asarray(current_perference, dtype=np.float32)[0]   # [B, H]
    W = np.asarray(W, dtype=np.float32)
    bvec = np.asarray(bvec, dtype=np.float32)
    lens = np.asarray(length_input).astype(np.int64)

    wt = np.ascontiguousarray(W.T)                             # [H(k), H(o)]
    bias = np.ascontiguousarray(bvec.reshape(H, 1))

    p = np.arange(128)[:, None]                                # [128, 1]
    t = np.arange(NT)[None, :]                                 # [1, NT]
    pos = (128 * t + p)                                        # [128, NT]

    in_maps = []
    for c in range(N_CORES):
        sl = slice(c * BPC, (c + 1) * BPC)
        mask = (pos[None, :, :] < lens[sl, None, None]).astype(np.float32)
        in_maps.append({
            "h": np.ascontiguousarray(h[sl]),
            "wt": wt,
            "usT": np.ascontiguousarray(us[sl].T),
            "bias": bias,
            "mask": np.ascontiguousarray(mask),
        })
    return in_maps


def run(short_perference, current_perference, W, b, length_input,
        trace=False, **run_kwargs):
    nc = _get_nc()
    in_maps = _prep_in_maps(short_perference, current_perference, W, b,
                            length_input)
    res = run_bass_kernel_spmd(nc, in_maps, list(range(N_CORES)),
                               trace=trace, **run_kwargs)
    outs = []
    for c in range(N_CORES):
        o = np.asarray(res.results[c]["out"], dtype=np.float32)  # [BPC, 241]
        outs.append(o[:, :H] / o[:, H:H + 1])
    v = np.concatenate(outs, axis=0)                             # [B, H]
    return v, res


def kernel(short_perference, current_perference, W, b, current_batch,
           length_input):
    v, _ = run(short_perference, current_perference, W, b, length_input)
    return v.astype(np.float32)
